# revision 1
# baseline (speedup 1.0000x reference)
"""Trainium2 Bass kernel for nn_CommunityModel (temporal community model).

Pipeline per gathered event-node row:
  s = state[idx] * exp(-softplus(log_decay) * max(t - last_t[idx], 0))
  p = softmax(relu(s @ W1 + b1) @ W2 + b2)

Strategy: data-parallel over the 8192-event batch across 8 NeuronCores
(1024 events -> 34816 gathered rows per core). The [100000, 256] state
table is packed host-side into 768-byte rows [last_t f32 | pad | state
bf16 x256 | pad] and replicated to every core's DRAM. Each core gathers
its rows with the GPSIMD dma_gather custom op (int16 indices =>
table bucketed into 4 shards of 25000 rows; events bucket-sorted by
shard on host, outputs unpermuted on host). Matmuls run in bf16 on the
PE: a gate-scaled-identity matmul doubles as the row->column transpose,
then W1 (2x2 chunked, PSUM-accumulated), relu+bias on ACT, W2, and a
batched softmax on DVE/ACT.
"""

import numpy as np
import ml_dtypes

N_NODES = 100000
D = 256
K = 5
B = 8192
R = 32
NCORES = 8
EV = B // NCORES              # events per core
MROWS = EV * (2 + R)          # gathered rows per core = 34816
NSH = 4                       # table shards (int16 index limit)
SHROWS = 25000                # rows per shard
EROW = 192                    # packed row: 192 f32 = 768 bytes
GROUP = 8                     # tiles (of 128 rows) per dma_gather call
SUP = 4                       # tiles per PE supertile

_cache = {}


def _build(tiles_per_shard):
    import concourse.bass as bass
    import concourse.tile as tile
    from concourse import bacc, mybir
    from concourse.masks import make_identity
    from concourse._compat import not_none as nn
    from bass_rust import ScopedClock

    # --- patch: split the Tile tail-drain multi-sem wait (walrus limit) ---
    def _drain_and_barrier(self, tick_clock, wait_clock):
        nc = self.nc
        drain_inst = nc.sync.drain()
        wait_clock.add_sem_waits(
            drain_inst.ins, ScopedClock({None: tick_clock.global_clock}))
        mi = drain_inst.ins
        waits = list(mi.sync_info.on_wait or [])
        if len(waits) > 1:
            mi.sync_info.on_wait = waits[:1]
            bb = nn(nc.cur_bb).bb
            insts = bb.instructions
            assert insts[-1] is mi
            insts.pop()
            for w in waits[1:]:
                nop = nc.sync.nop(nofuse=True, hint='drain_split_wait')
                nop.ins.sync_info = mybir.SyncInfo(on_wait=[w], on_update=[])
            insts.append(mi)
        nc.all_engine_barrier()
        assert self.sems is not None
        popped = nc._tile_sem_poison_stack.pop()
        assert popped is self._sem_poison
        nc.clear_and_free_semaphores(list(self.sems.allocated().values()))
        nc.all_engine_barrier()

    tile.TileContext._drain_and_barrier = _drain_and_barrier

    f32, bf16, i16 = mybir.dt.float32, mybir.dt.bfloat16, mybir.dt.int16
    TOT_TILES = sum(tiles_per_shard)

    # static group list: (shard, tile_base, n_tiles)
    groups = []
    tb = 0
    for s in range(NSH):
        left = tiles_per_shard[s]
        while left:
            nt = min(GROUP, left)
            groups.append((s, tb, nt))
            tb += nt
            left -= nt

    nc = bacc.Bacc('TRN2', target_bir_lowering=False, debug=False)
    table = nc.dram_tensor('table', [N_NODES, EROW], f32, kind='ExternalInput')
    idx_d = nc.dram_tensor('idx', [128, TOT_TILES * 8], i16, kind='ExternalInput')
    t_d = nc.dram_tensor('t', [128, TOT_TILES], f32, kind='ExternalInput')
    w1_d = nc.dram_tensor('w1', [128, 2, D], bf16, kind='ExternalInput')
    w2_d = nc.dram_tensor('w2', [128, 2, K], bf16, kind='ExternalInput')
    b1_d = nc.dram_tensor('b1', [128, 2], f32, kind='ExternalInput')
    b2_d = nc.dram_tensor('b2r', [128, GROUP * K], f32, kind='ExternalInput')
    ngc_d = nc.dram_tensor('negc', [128, 1], f32, kind='ExternalInput')
    y = nc.dram_tensor('y', [128, TOT_TILES * K], f32, kind='ExternalOutput')

    with tile.TileContext(nc) as tc:
        with (
            tc.tile_pool(name='const', bufs=1) as cpool,
            tc.tile_pool(name='gbuf', bufs=2) as gpool,
            tc.tile_pool(name='gid', bufs=2) as gidpool,
            tc.tile_pool(name='small', bufs=3) as spool,
            tc.tile_pool(name='xts', bufs=3) as xtspool,
            tc.tile_pool(name='hts', bufs=3) as htspool,
            tc.tile_pool(name='soft', bufs=2) as softpool,
            tc.tile_pool(name='xtp', bufs=2, space='PSUM') as xtppool,
            tc.tile_pool(name='htp', bufs=1, space='PSUM') as htppool,
            tc.tile_pool(name='lgp', bufs=2, space='PSUM') as lgppool,
        ):
            idx_sb = cpool.tile([128, TOT_TILES * 8], i16)
            nc.sync.dma_start(idx_sb[:], idx_d[:])
            t_sb = cpool.tile([128, TOT_TILES], f32)
            nc.sync.dma_start(t_sb[:], t_d[:])
            w1_sb = cpool.tile([128, 2, D], bf16)
            nc.sync.dma_start(w1_sb[:], w1_d[:])
            w2_sb = cpool.tile([128, 2, K], bf16)
            nc.sync.dma_start(w2_sb[:], w2_d[:])
            b1_sb = cpool.tile([128, 2], f32)
            nc.sync.dma_start(b1_sb[:], b1_d[:])
            b2_sb = cpool.tile([128, GROUP * K], f32)
            nc.sync.dma_start(b2_sb[:], b2_d[:])
            ngc_sb = cpool.tile([128, 1], f32)
            nc.sync.dma_start(ngc_sb[:], ngc_d[:])
            ident = cpool.tile([128, 128], f32)
            make_identity(nc, ident[:])

            for (s, tbase, nt) in groups:
                nidx = nt * 128
                gb = gpool.tile([128, GROUP * EROW], f32, tag='gb')
                gb3 = gb[:, :nt * EROW].rearrange('p (t d) -> p t d', d=EROW)
                nc.gpsimd.dma_gather(
                    out_ap=gb3,
                    in_ap=table[s * SHROWS:(s + 1) * SHROWS, :],
                    idxs_ap=idx_sb[:, tbase * 8:(tbase + nt) * 8],
                    num_idxs=nidx,
                    num_idxs_reg=nidx,
                    elem_size=EROW,
                )
                gbb = gb[:, :nt * EROW].bitcast(bf16).rearrange(
                    'p (t d) -> p t d', d=2 * EROW)

                # gate = exp(-c * max(t - last_t, 0))   [128, nt]
                lt = gb3[:, :, 0:1].rearrange('p t o -> p (t o)')
                dt_t = spool.tile([128, GROUP], f32, tag='dt')
                nc.vector.tensor_tensor(
                    out=dt_t[:, :nt], in0=t_sb[:, tbase:tbase + nt], in1=lt,
                    op=mybir.AluOpType.subtract)
                nc.vector.tensor_scalar_max(dt_t[:, :nt], dt_t[:, :nt], 0.0)
                gate = spool.tile([128, GROUP], f32, tag='gate')
                nc.scalar.activation(
                    gate[:, :nt], dt_t[:, :nt],
                    mybir.ActivationFunctionType.Exp, scale=ngc_sb[:, 0:1])

                # gate-scaled identity blocks (bf16)
                gid = gidpool.tile([128, GROUP, 128], bf16, tag='gid')
                for j in range(nt):
                    nc.vector.tensor_scalar_mul(
                        gid[:, j, :], ident[:], gate[:, j:j + 1])

                lg = lgppool.tile([128, GROUP * K], f32, tag='lg')

                for c0 in range(0, nt, SUP):
                    ns = min(SUP, nt - c0)
                    nw = ns * 128
                    xt_ps = xtppool.tile([128, 2, 512], f32, tag='xtp')
                    for j in range(ns):
                        for f in range(2):
                            # xT chunk = S_j^T @ diag(gate_j)
                            nc.tensor.matmul(
                                xt_ps[:, f, j * 128:(j + 1) * 128],
                                lhsT=gbb[:, c0 + j, 4 + 128 * f:132 + 128 * f],
                                rhs=gid[:, c0 + j, :],
                                start=True, stop=True)
                    xt_sb = xtspool.tile([128, 2, 512], bf16, tag='xts')
                    nc.vector.tensor_copy(xt_sb[:, 0, :nw], xt_ps[:, 0, :nw])
                    nc.scalar.copy(xt_sb[:, 1, :nw], xt_ps[:, 1, :nw])

                    ht_ps = htppool.tile([128, 2, 512], f32, tag='htp')
                    for o in range(2):
                        for f in range(2):
                            nc.tensor.matmul(
                                ht_ps[:, o, :nw],
                                lhsT=w1_sb[:, f, o * 128:(o + 1) * 128],
                                rhs=xt_sb[:, f, :nw],
                                start=(f == 0), stop=(f == 1))
                    ht_sb = htspool.tile([128, 2, 512], bf16, tag='hts')
                    for o in range(2):
                        nc.scalar.activation(
                            ht_sb[:, o, :nw], ht_ps[:, o, :nw],
                            mybir.ActivationFunctionType.Relu,
                            bias=b1_sb[:, o:o + 1])

                    for j in range(ns):
                        jj = c0 + j
                        for o in range(2):
                            nc.tensor.matmul(
                                lg[:, jj * K:(jj + 1) * K],
                                lhsT=ht_sb[:, o, j * 128:(j + 1) * 128],
                                rhs=w2_sb[:, o, :],
                                start=(o == 0), stop=(o == 1))

                # softmax over K for all nt tiles
                e_t = softpool.tile([128, GROUP * K], f32, tag='e')
                nc.vector.tensor_tensor(
                    out=e_t[:, :nt * K], in0=lg[:, :nt * K],
                    in1=b2_sb[:, :nt * K], op=mybir.AluOpType.add)
                nc.scalar.activation(
                    e_t[:, :nt * K], e_t[:, :nt * K],
                    mybir.ActivationFunctionType.Exp)
                ssum = spool.tile([128, GROUP], f32, tag='ssum')
                nc.vector.tensor_reduce(
                    ssum[:, :nt],
                    e_t[:, :nt * K].rearrange('p (t k) -> p t k', k=K),
                    axis=mybir.AxisListType.X, op=mybir.AluOpType.add)
                rec = spool.tile([128, GROUP], f32, tag='rec')
                nc.vector.reciprocal(rec[:, :nt], ssum[:, :nt])
                pr = softpool.tile([128, GROUP * K], f32, tag='pr')
                for j in range(nt):
                    nc.vector.tensor_scalar_mul(
                        pr[:, j * K:(j + 1) * K], e_t[:, j * K:(j + 1) * K],
                        rec[:, j:j + 1])
                nc.sync.dma_start(
                    y[:, tbase * K:(tbase + nt) * K], pr[:, :nt * K])

    nc.compile()
    return nc, groups, TOT_TILES


def kernel(src, dst, neg, ts, edge_idxs, state, last_t, log_decay,
           W1, b1, W2, b2):
    from concourse.bass_utils import run_bass_kernel_spmd

    bf = ml_dtypes.bfloat16
    src = np.asarray(src).astype(np.int64)
    dst = np.asarray(dst).astype(np.int64)
    neg = np.asarray(neg).astype(np.int64)
    ts = np.asarray(ts, dtype=np.float32)
    state = np.asarray(state, dtype=np.float32)
    last_t = np.asarray(last_t, dtype=np.float32)
    W1 = np.asarray(W1, dtype=np.float32)
    b1 = np.asarray(b1, dtype=np.float32)
    W2 = np.asarray(W2, dtype=np.float32)
    b2 = np.asarray(b2, dtype=np.float32)
    c = float(np.log1p(np.exp(np.float64(np.asarray(log_decay)))))

    # ---- packed table: [lt f32 | pad f32 | 256 x bf16 | pad] = 192 f32 ----
    tab = np.zeros((N_NODES, EROW), dtype=np.float32)
    tab[:, 0] = last_t
    tab_u16 = tab.view(np.uint16).reshape(N_NODES, 2 * EROW)
    tab_u16[:, 4:4 + D] = state.astype(bf).view(np.uint16)

    # ---- per-core event bucketing ----
    idx_all = np.concatenate(
        [src.reshape(NCORES, EV), dst.reshape(NCORES, EV),
         neg.reshape(NCORES, EV * R)], axis=1).astype(np.int64)  # [NC, MROWS]
    t_all = np.concatenate(
        [ts.reshape(NCORES, EV), ts.reshape(NCORES, EV),
         np.repeat(ts.reshape(NCORES, EV), R, axis=1)], axis=1)  # [NC, MROWS]

    shard = idx_all // SHROWS
    orders = [np.argsort(shard[cid], kind='stable') for cid in range(NCORES)]
    counts = np.stack([np.bincount(shard[cid], minlength=NSH)
                       for cid in range(NCORES)])  # [NC, NSH]
    caps = ((counts.max(axis=0) + 127) // 128 * 128).astype(int)
    tiles_per_shard = tuple(int(x) // 128 for x in caps)

    key = tiles_per_shard
    if key not in _cache:
        _cache[key] = _build(tiles_per_shard)
    nc, groups, TOT_TILES = _cache[key]

    # ---- per-core slot layout ----
    in_maps = []
    slot_maps = []
    w1_h = np.ascontiguousarray(
        W1.reshape(2, 128, D).transpose(1, 0, 2)).astype(bf)
    w2_h = np.ascontiguousarray(
        W2.reshape(2, 128, K).transpose(1, 0, 2)).astype(bf)
    b1_h = np.ascontiguousarray(b1.reshape(2, 128).T)
    b2_h = np.ascontiguousarray(np.tile(b2, (128, GROUP)))
    ngc_h = np.full((128, 1), -c, dtype=np.float32)

    for cid in range(NCORES):
        order = orders[cid]
        cnt = counts[cid]
        nslots = TOT_TILES * 128
        loc_idx = np.zeros(nslots, dtype=np.int16)
        t_slot = np.zeros(nslots, dtype=np.float32)
        slot_ev = np.full(nslots, -1, dtype=np.int64)
        base = 0
        pos = 0
        for s in range(NSH):
            evs = order[pos:pos + cnt[s]]
            pos += cnt[s]
            sl = slice(base, base + len(evs))
            loc_idx[sl] = (idx_all[cid, evs] - s * SHROWS).astype(np.int16)
            t_slot[sl] = t_all[cid, evs]
            slot_ev[sl] = evs
            base += caps[s]
        # wrapped int16 index layout per group
        idx16 = np.zeros((128, TOT_TILES * 8), dtype=np.int16)
        for (s, tbase, nt) in groups:
            v = loc_idx[tbase * 128:(tbase + nt) * 128]
            idx16[:, tbase * 8:(tbase + nt) * 8] = np.tile(
                v.reshape(nt * 8, 16).T, (8, 1))
        t2d = np.ascontiguousarray(t_slot.reshape(TOT_TILES, 128).T)
        in_maps.append({
            'table': tab, 'idx': idx16, 't': t2d, 'w1': w1_h, 'w2': w2_h,
            'b1': b1_h, 'b2r': b2_h, 'negc': ngc_h,
        })
        slot_maps.append(slot_ev)

    res = run_bass_kernel_spmd(nc, in_maps, core_ids=list(range(NCORES)))

    p_src = np.empty((B, K), np.float32)
    p_dst = np.empty((B, K), np.float32)
    p_neg = np.empty((B, R, K), np.float32)
    for cid in range(NCORES):
        yv = res.results[cid]['y'].reshape(128, TOT_TILES, K)
        probs = yv.transpose(1, 0, 2).reshape(-1, K)
        ev_out = np.empty((MROWS, K), np.float32)
        valid = slot_maps[cid] >= 0
        ev_out[slot_maps[cid][valid]] = probs[valid]
        e0 = cid * EV
        p_src[e0:e0 + EV] = ev_out[:EV]
        p_dst[e0:e0 + EV] = ev_out[EV:2 * EV]
        p_neg[e0:e0 + EV] = ev_out[2 * EV:].reshape(EV, R, K)
    return p_src, p_dst, p_neg
